# revision 30
# baseline (speedup 1.0000x reference)
"""Cross-attention Bass/Tile kernel for Trainium2, sharded over 8 NeuronCores.

Problem (fixed shapes): B=2, T=2048, C=1024, H=16 heads, D=64.
    q = x_q @ Wq + bq;  kv = x_kv @ Wkv + bkv;  k, v = split(kv)
    y = softmax(q k^T / sqrt(D)) v;  out = y @ Wo + bo

Sharding: 8 cores = 2 (batch) x 4 (head groups of 4 heads, 256 channels).
Each core computes its head-group's projections + attention + a partial
output projection (its 256 rows of Wo); the host sums the 4 partials per
batch.  The v-bias and output bias are folded in exactly on the host:
    y = att@(V + 1*bv) = att@V + 1*bv   (att rows sum to 1)
    => out += bv @ Wo + bo              (added once per batch on the host)

v3 (over the f32r baseline):
  - bf16 operands everywhere (x and weights staged bf16 from host):
    halves DMA traffic; psum stays f32.
  - x^T via DMA-transpose (XBAR, 16x128 tiles) straight from DRAM to
    SBUF: eliminates all PE transposes (~49k cycles) and the DVE
    psum->SBUF copy-outs (~34us).
  - K projection at 512-token granularity (64 instead of 128 matmuls).
  - Output partials stored bf16 (halves store DMA).
  - Phase A (K/V prep) streams into the first attention pass as woven
    units instead of a serial prologue.

Attention per (tq 512-block, head-pair) pass, per tk chunk: S^T matmul
(2 heads row-packed via tile_position) -> exp on ACT (scale=1/8) ->
att@V matmuls lagging LAG units.  V carries a ones column so row 64 of
the y psum accumulates the softmax denominator; normalization is
reciprocal + K=1 broadcast matmul + DVE multiply (baseline-proven).
PE matmul count kept low (~850): the PE sequencer costs ~130ns per
instruction (SW decode), which is the binding constraint before engine
cycles for narrow matmuls.

PSUM (8 banks): 2 x [128,1024] "s" + 4 x [128,512] "y" slots shared by
y-accumulators and woven work units (baseline-proven rotation).
"""

import numpy as np

B = 2
T = 2048
C = 1024
H = 16
D = 64
NCORES = 8
TPG = 4  # tensor-parallel group size (head groups)
HL = H // TPG  # heads per core = 4
CL = HL * D  # local channels = 256
P = 128

_CACHE = {}


def _build(debug=False):
    import concourse.tile as tile
    from concourse import bacc, mybir

    f32 = mybir.dt.float32
    bf16 = mybir.dt.bfloat16
    Exp = mybir.ActivationFunctionType.Exp

    nc = bacc.Bacc("TRN2", target_bir_lowering=False, debug=False)

    xq_d = nc.dram_tensor("xq", [T, C], bf16, kind="ExternalInput")
    xkv_d = nc.dram_tensor("xkv", [T, C], bf16, kind="ExternalInput")
    # weights prepacked on host into three bf16 blobs (DMA chain order):
    # wb1=[wq 8x256 | bq 2 | bk 2], wkv=[wk 8x256 | wv 8x256], wo=[2x1024]
    wb1_d = nc.dram_tensor("wb1", [P, 2052], bf16, kind="ExternalInput")
    wkv_d = nc.dram_tensor("wkv", [P, 4096], bf16, kind="ExternalInput")
    wo_d = nc.dram_tensor("wo", [P, 2048], bf16, kind="ExternalInput")
    out_d = nc.dram_tensor("out", [T, C], bf16, kind="ExternalOutput")

    KC = C // P  # 8 contraction chunks for the projections
    NT = T // P  # 16 token chunks of 128
    NQ = 4  # tq blocks of 512
    QW = T // NQ  # 512
    DC = CL // P  # 2 chunks of d_local
    LAG = 4

    with tile.TileContext(nc) as tc:
        with (
            tc.tile_pool(name="const", bufs=1) as const,
            tc.tile_pool(name="persist", bufs=1) as persist,
            tc.tile_pool(name="ework", bufs=7) as ework,
            tc.tile_pool(name="norm2", bufs=1) as norm2,
            tc.tile_pool(name="outst", bufs=3) as outst,
        ):
            from concourse.masks import make_identity

            identf = const.tile([P, P], f32)
            make_identity(nc, identf)
            identb = const.tile([P, P], bf16)
            nc.vector.tensor_copy(identb, identf)
            ones4 = const.tile([P, HL, 1], bf16)
            nc.vector.memset(ones4, 1.0)
            onesb = const.tile([P, 64], bf16)
            nc.vector.memset(onesb, 1.0)

            # ---- weights: ONE blob DMA + one bias DMA (DMA instructions
            # issue serially at ~2.7us each; count is precious) ----
            wb1_sb = const.tile([P, 2052], bf16)
            nc.gpsimd.dma_start(wb1_sb, wb1_d[:, :])
            wkv_sb = const.tile([P, 4096], bf16)
            wo_sb = const.tile([P, 2048], bf16)
            bias_f = const.tile([P, 4], f32)
            nc.vector.tensor_copy(bias_f, wb1_sb[:, 2048:2052])
            bq_sb = bias_f[:, 0:2]
            bk_sb = bias_f[:, 2:4]

            def wq_ap(kc, sl):
                return wb1_sb[:, kc * CL + sl.start : kc * CL + sl.stop]

            def wk_ap(kc, sl):
                return wkv_sb[:, kc * CL + sl.start : kc * CL + sl.stop]

            def wv_ap(kc):
                return wkv_sb[:, 2048 + kc * CL : 2048 + (kc + 1) * CL]

            def wo_ap(dc, sl):
                return wo_sb[:, dc * C + sl.start : dc * C + sl.stop]

            # ---- persistent activations ----
            xq_t = persist.tile([P, KC, T], bf16)  # xq^T  [c, t]
            xkv_t = persist.tile([P, KC, T], bf16)  # xkv^T [c, t]
            qt_sb = persist.tile([P, DC, T], bf16)  # Q^T  [d, t]
            kt_sb = persist.tile([P, DC, T], bf16)  # K^T  [d, t]
            v_sb = persist.tile([P, NT, HL, 66], bf16)  # V|1 [t, h, d+1]
            yt_sb = persist.tile([P, DC, T], bf16)  # y^T  [d, t] (normalized)

            # ---- input transposes (XBAR DMA): ONE [512,1024] DMA per
            # granule covers all 8 c-chunks -> out[p, c, t] = x^T[c*128+p, t]
            def emit_xT(dst, src_d, g):
                t0 = g * QW
                nc.sync.dma_start(
                    dst[:, :, t0 : t0 + QW],
                    src_d[t0 : t0 + QW, :],
                    transpose=True,
                )

            emit_xT(xq_t, xq_d, 0)
            nc.gpsimd.dma_start(wkv_sb, wkv_d[:, :])
            emit_xT(xkv_t, xkv_d, 0)
            emit_xT(xkv_t, xkv_d, 1)
            nc.gpsimd.dma_start(wo_sb, wo_d[:, :])
            for g in range(2, NQ):
                emit_xT(xkv_t, xkv_d, g)
            for g in range(1, NQ):
                emit_xT(xq_t, xq_d, g)

            # ---- kernel-wide PSUM: 2 x [128,1024] (s) + 4 x [128,512] (y)
            ps_s = tc.alloc_tile_pool(name="ps_s", bufs=2, space="PSUM")
            ps_y = tc.alloc_tile_pool(name="ps_y", bufs=4, space="PSUM")

            # ---------- emission helpers ----------
            def vproj_unit(tch):
                def u():
                    pv = ps_y.tile([P, QW], f32, tag="y", name="pv")
                    for c in range(KC):
                        nc.tensor.matmul(
                            pv[:, :CL],
                            xkv_t[:, c, tch * P : (tch + 1) * P],
                            wv_ap(c),
                            start=(c == 0),
                            stop=(c == KC - 1),
                        )
                    nc.vector.tensor_copy(
                        v_sb[:, tch, :, 0:64],
                        pv[:, :CL].rearrange("p (h d) -> p h d", h=HL),
                    )
                    nc.vector.tensor_copy(v_sb[:, tch, :, 64:65], ones4)

                return u

            def kproj_unit(g, dc):
                def u():
                    pp = ps_y.tile([P, QW], f32, tag="y", name="ppk")
                    for c in range(KC):
                        nc.tensor.matmul(
                            pp,
                            wk_ap(c, slice(dc * P, (dc + 1) * P)),
                            xkv_t[:, c, g * QW : (g + 1) * QW],
                            start=(c == 0),
                            stop=(c == KC - 1),
                        )
                    nc.vector.tensor_scalar_add(
                        kt_sb[:, dc, g * QW : (g + 1) * QW],
                        pp,
                        bk_sb[:, dc : dc + 1],
                    )

                return u

            def q_prep_units(tq):
                units = []
                for dc in range(DC):

                    def proj_u(dc=dc):
                        pp = ps_y.tile([P, QW], f32, tag="y", name="ppq")
                        for c in range(KC):
                            nc.tensor.matmul(
                                pp,
                                wq_ap(c, slice(dc * P, (dc + 1) * P)),
                                xq_t[:, c, tq * QW : (tq + 1) * QW],
                                start=(c == 0),
                                stop=(c == KC - 1),
                            )
                        nc.vector.tensor_scalar_add(
                            qt_sb[:, dc, tq * QW : (tq + 1) * QW],
                            pp,
                            bq_sb[:, dc : dc + 1],
                        )

                    units.append(proj_u)
                return units

            out_po = out_d.rearrange("(k f p) c -> k p f c", p=P, f=4)
            out_pq = out_d.rearrange("(t p) c -> t p c", p=P)

            def po_units(tq):
                units = []
                state = {}
                for ts_ in range(4):
                    tch = tq * 4 + ts_
                    for co in range(2):

                        def u(tch=tch, ts_=ts_, co=co):
                            if "o" not in state:
                                state["o"] = outst.tile([P, 4, C], bf16, tag="o", name="o_st")
                            po = ps_y.tile([P, QW], f32, tag="y", name="po")
                            for dc in range(DC):
                                nc.tensor.matmul(
                                    po,
                                    yt_sb[:, dc, tch * P : (tch + 1) * P],
                                    wo_ap(dc, slice(co * QW, (co + 1) * QW)),
                                    start=(dc == 0),
                                    stop=(dc == DC - 1),
                                )
                            nc.vector.tensor_copy(
                                state["o"][:, ts_, co * QW : (co + 1) * QW], po
                            )
                            if tq == NQ - 1:
                                if co == 1:
                                    nc.sync.dma_start(
                                        out_pq[tch], state["o"][:, ts_, :]
                                    )
                            elif ts_ == 3 and co == 1:
                                nc.sync.dma_start(out_po[tq], state["o"])

                        units.append(u)
                return units

            # phase-A prep as a streamable queue: per granule g (512 tok):
            # 4 V-proj chunks + 2 K-proj halves
            prep_q = []
            for g in range(NQ):
                for ts_ in range(4):
                    prep_q.append(vproj_unit(g * 4 + ts_))
                for dc in range(DC):
                    prep_q.append(kproj_unit(g, dc))

            # ---- phase B: attention passes per (tq, head-pair) ----
            y_tiles = {}
            e_tiles = {}

            def emit_sexp(k, hc, tk):
                sp = ps_s.tile([P, 2 * QW], f32, tag="s", name="sp")
                for hh in range(2):
                    nc.tensor.matmul(
                        sp[:, hh * QW : (hh + 1) * QW],
                        kt_sb[hh * 64 : (hh + 1) * 64, hc, tk * P : (tk + 1) * P],
                        qt_sb[hh * 64 : (hh + 1) * 64, hc, k * QW : (k + 1) * QW],
                        start=True,
                        stop=True,
                        tile_position=(hh * 64, 0),
                    )
                e2 = ework.tile([P, 2 * QW], bf16, tag="e", name="e2")
                nc.scalar.activation(e2, sp, Exp, scale=0.125)
                e_tiles[(k, hc, tk)] = e2

            def emit_y(k, hc, tk):
                if (k, hc) not in y_tiles:
                    y_tiles[(k, hc)] = [
                        ps_y.tile([65, QW], f32, tag="y", name=f"y_ps{i}")
                        for i in range(2)
                    ]
                y_pair = y_tiles[(k, hc)]
                e2 = e_tiles.pop((k, hc, tk))
                for hh in range(2):
                    h = 2 * hc + hh
                    nc.tensor.matmul(
                        y_pair[hh],
                        v_sb[:, tk, h, :65],
                        e2[:, hh * QW : (hh + 1) * QW],
                        start=(tk == 0),
                        stop=(tk == NT - 1),
                    )

            def emit_norm(k, hc):
                y_pair = y_tiles.pop((k, hc))
                recr = norm2.tile([P, 2, QW], bf16, tag="recr")
                with nc.allow_low_precision(reason="softmax denom reciprocal"):
                    for hh in range(2):
                        nc.vector.reciprocal(
                            recr[64:65, hh, :], y_pair[hh][64:65, :]
                        )
                rbp = ps_s.tile([P, 2 * QW], f32, tag="s", name="rbp")
                for hh in range(2):
                    nc.tensor.matmul(
                        rbp[0:64, hh * QW : (hh + 1) * QW],
                        onesb[64:65, :],
                        recr[64:65, hh, :],
                        start=True,
                        stop=True,
                        tile_position=(64, 0),
                        skip_group_check=True,
                    )
                rbs = norm2.tile([P, 2 * QW], f32, tag="rbs")
                nc.vector.tensor_copy(rbs[0:64, :], rbp[0:64, :])
                for hh in range(2):
                    rb_h = rbs[0:64, hh * QW : (hh + 1) * QW]
                    if hh == 0:
                        nc.vector.tensor_mul(
                            out=yt_sb[0:64, hc, k * QW : (k + 1) * QW],
                            in0=y_pair[hh][0:64, :],
                            in1=rb_h,
                        )
                    else:
                        yst = norm2.tile([64, QW], bf16, tag="yst")
                        nc.vector.tensor_mul(
                            out=yst, in0=y_pair[hh][0:64, :], in1=rb_h
                        )
                        ytp = ps_y.tile([P, QW], f32, tag="y", name="ytp")
                        nc.tensor.matmul(
                            ytp[64:128, :],
                            identb[0:64, 0:64],
                            yst,
                            start=True,
                            stop=True,
                            tile_position=(0, 64),
                            skip_group_check=True,
                        )
                        nc.vector.tensor_copy(
                            yt_sb[64:128, hc, k * QW : (k + 1) * QW],
                            ytp[64:128, :],
                        )

            passes = [(k, hc) for k in range(NQ) for hc in range(DC)]
            unit_q = []
            yq = []
            # Q-prep for tq0 first (xq g0 is the first DMA), then granule-0
            # prep so S(0,0,0) has kt/v chunk 0
            for u in q_prep_units(0):
                u()
            for _ in range(6):
                prep_q.pop(0)()

            for pi, (k, hc) in enumerate(passes):
                if hc == 0 and k + 1 < NQ:
                    unit_q.extend(q_prep_units(k + 1))
                for tk in range(NT):
                    emit_sexp(k, hc, tk)
                    yq.append((k, hc, tk))
                    if len(yq) > LAG:
                        emit_y(*yq.pop(0))
                    if tk == 1 and pi >= 1:
                        pk, phc = passes[pi - 1]
                        while yq and yq[0][:2] == (pk, phc):
                            emit_y(*yq.pop(0))
                        emit_norm(pk, phc)
                        if hc == 0 and k >= 1:
                            unit_q.extend(po_units(k - 1))
                    # stream phase-A prep ahead of need during the first pass
                    if prep_q:
                        prep_q.pop(0)()
                        if tk % 2 == 0 and prep_q and pi == 0:
                            prep_q.pop(0)()
                    elif unit_q:
                        unit_q.pop(0)()
            while unit_q:
                unit_q.pop(0)()
            while yq:
                emit_y(*yq.pop(0))
            emit_norm(NQ - 1, DC - 1)
            for u in po_units(NQ - 1):
                u()

            ps_y.release()
            ps_s.release()

    nc.compile()
    return nc


def _get_nc():
    if "nc" not in _CACHE:
        _CACHE["nc"] = _build()
    return _CACHE["nc"]


def _shard_inputs(x_q, x_kv, Wq, bq, Wkv, bkv, Wo):
    import ml_dtypes

    bf16 = ml_dtypes.bfloat16

    def pack_proj(W):  # [C, CL] -> [128, 8*256] in (kc, d) order
        return W.reshape(8, P, CL).transpose(1, 0, 2).reshape(P, 8 * CL)

    in_maps = []
    for core in range(NCORES):
        b = core // TPG
        g = core % TPG
        cols = slice(g * CL, (g + 1) * CL)
        wo_loc = Wo[g * CL : (g + 1) * CL, :]  # [256, 1024]
        wblob1 = np.concatenate(
            [
                pack_proj(Wq[:, cols]),
                bq[cols].reshape(2, P).T,
                bkv[:C][cols].reshape(2, P).T,
            ],
            axis=1,
        )
        wkvblob = np.concatenate(
            [
                pack_proj(Wkv[:, :C][:, cols]),
                pack_proj(Wkv[:, C:][:, cols]),
            ],
            axis=1,
        )
        woblob = wo_loc.reshape(2, P, C).transpose(1, 0, 2).reshape(P, 2 * C)
        in_maps.append(
            {
                "xq": np.ascontiguousarray(x_q[b]).astype(bf16),
                "xkv": np.ascontiguousarray(x_kv[b]).astype(bf16),
                "wb1": np.ascontiguousarray(wblob1).astype(bf16),
                "wkv": np.ascontiguousarray(wkvblob).astype(bf16),
                "wo": np.ascontiguousarray(woblob).astype(bf16),
            }
        )
    return in_maps


def kernel(x_q, x_kv, Wq, bq, Wkv, bkv, Wo, bo):
    from concourse.bass_utils import run_bass_kernel_spmd

    x_q = np.asarray(x_q, dtype=np.float32)
    x_kv = np.asarray(x_kv, dtype=np.float32)
    Wq = np.asarray(Wq, dtype=np.float32)
    bq = np.asarray(bq, dtype=np.float32)
    Wkv = np.asarray(Wkv, dtype=np.float32)
    bkv = np.asarray(bkv, dtype=np.float32)
    Wo = np.asarray(Wo, dtype=np.float32)
    bo = np.asarray(bo, dtype=np.float32)

    nc = _get_nc()
    in_maps = _shard_inputs(x_q, x_kv, Wq, bq, Wkv, bkv, Wo)
    res = run_bass_kernel_spmd(nc, in_maps, core_ids=list(range(NCORES)))

    # host-side gather: sum tensor-parallel partials; add exact bias terms
    bias_full = bkv[C:] @ Wo + bo  # v-bias through Wo, plus output bias
    out = np.zeros((B, T, C), dtype=np.float32)
    for core in range(NCORES):
        out[core // TPG] += np.asarray(res.results[core]["out"]).astype(np.float32)
    out += bias_full[None, None, :]
    return out


# revision 31
# speedup vs baseline: 1.0305x; 1.0305x over previous
"""Cross-attention Bass/Tile kernel for Trainium2, sharded over 8 NeuronCores.

Problem (fixed shapes): B=2, T=2048, C=1024, H=16 heads, D=64.
    q = x_q @ Wq + bq;  kv = x_kv @ Wkv + bkv;  k, v = split(kv)
    y = softmax(q k^T / sqrt(D)) v;  out = y @ Wo + bo

Sharding: 8 cores = 2 (batch) x 4 (head groups of 4 heads, 256 channels).
Each core computes its head-group's projections + attention + a partial
output projection (its 256 rows of Wo); the host sums the 4 partials per
batch.  The v-bias and output bias are folded in exactly on the host:
    y = att@(V + 1*bv) = att@V + 1*bv   (att rows sum to 1)
    => out += bv @ Wo + bo              (added once per batch on the host)

v3 (over the f32r baseline):
  - bf16 operands everywhere (x and weights staged bf16 from host):
    halves DMA traffic; psum stays f32.
  - x^T via DMA-transpose (XBAR, 16x128 tiles) straight from DRAM to
    SBUF: eliminates all PE transposes (~49k cycles) and the DVE
    psum->SBUF copy-outs (~34us).
  - K projection at 512-token granularity (64 instead of 128 matmuls).
  - Output partials stored bf16 (halves store DMA).
  - Phase A (K/V prep) streams into the first attention pass as woven
    units instead of a serial prologue.

Attention per (tq 512-block, head-pair) pass, per tk chunk: S^T matmul
(2 heads row-packed via tile_position) -> exp on ACT (scale=1/8) ->
att@V matmuls lagging LAG units.  V carries a ones column so row 64 of
the y psum accumulates the softmax denominator; normalization is
reciprocal + K=1 broadcast matmul + DVE multiply (baseline-proven).
PE matmul count kept low (~850): the PE sequencer costs ~130ns per
instruction (SW decode), which is the binding constraint before engine
cycles for narrow matmuls.

PSUM (8 banks): 2 x [128,1024] "s" + 4 x [128,512] "y" slots shared by
y-accumulators and woven work units (baseline-proven rotation).
"""

import numpy as np

B = 2
T = 2048
C = 1024
H = 16
D = 64
NCORES = 8
TPG = 4  # tensor-parallel group size (head groups)
HL = H // TPG  # heads per core = 4
CL = HL * D  # local channels = 256
P = 128

_CACHE = {}


def _build(debug=False):
    import concourse.tile as tile
    from concourse import bacc, mybir

    f32 = mybir.dt.float32
    bf16 = mybir.dt.bfloat16
    Exp = mybir.ActivationFunctionType.Exp

    nc = bacc.Bacc("TRN2", target_bir_lowering=False, debug=False)

    xq_d = nc.dram_tensor("xq", [T, C], bf16, kind="ExternalInput")
    xkv_d = nc.dram_tensor("xkv", [T, C], bf16, kind="ExternalInput")
    # weights prepacked on host into three bf16 blobs (DMA chain order):
    # wb1=[wq 8x256 | bq 2 | bk 2], wkv=[wk 8x256 | wv 8x256], wo=[2x1024]
    wb1_d = nc.dram_tensor("wb1", [P, 2052], bf16, kind="ExternalInput")
    wkv_d = nc.dram_tensor("wkv", [P, 4096], bf16, kind="ExternalInput")
    wo_d = nc.dram_tensor("wo", [P, 2048], bf16, kind="ExternalInput")
    out_d = nc.dram_tensor("out", [T, C], bf16, kind="ExternalOutput")

    KC = C // P  # 8 contraction chunks for the projections
    NT = T // P  # 16 token chunks of 128
    NQ = 4  # tq blocks of 512
    QW = T // NQ  # 512
    DC = CL // P  # 2 chunks of d_local
    LAG = 4

    with tile.TileContext(nc) as tc:
        with (
            tc.tile_pool(name="const", bufs=1) as const,
            tc.tile_pool(name="persist", bufs=1) as persist,
            tc.tile_pool(name="ework", bufs=7) as ework,
            tc.tile_pool(name="norm2", bufs=1) as norm2,
            tc.tile_pool(name="outst", bufs=3) as outst,
        ):
            from concourse.masks import make_identity

            identf = const.tile([P, P], f32)
            make_identity(nc, identf)
            identb = const.tile([P, P], bf16)
            nc.vector.tensor_copy(identb, identf)
            ones4 = const.tile([P, HL, 1], bf16)
            nc.vector.memset(ones4, 1.0)
            onesb = const.tile([P, 64], bf16)
            nc.vector.memset(onesb, 1.0)

            # ---- weights: ONE blob DMA + one bias DMA (DMA instructions
            # issue serially at ~2.7us each; count is precious) ----
            wb1_sb = const.tile([P, 2052], bf16)
            nc.gpsimd.dma_start(wb1_sb, wb1_d[:, :])
            wkv_sb = const.tile([P, 4096], bf16)
            wo_sb = const.tile([P, 2048], bf16)
            bias_f = const.tile([P, 4], f32)
            nc.vector.tensor_copy(bias_f, wb1_sb[:, 2048:2052])
            bq_sb = bias_f[:, 0:2]
            bk_sb = bias_f[:, 2:4]

            def wq_ap(kc, sl):
                return wb1_sb[:, kc * CL + sl.start : kc * CL + sl.stop]

            def wk_ap(kc, sl):
                return wkv_sb[:, kc * CL + sl.start : kc * CL + sl.stop]

            def wv_ap(kc):
                return wkv_sb[:, 2048 + kc * CL : 2048 + (kc + 1) * CL]

            def wo_ap(dc, sl):
                return wo_sb[:, dc * C + sl.start : dc * C + sl.stop]

            # ---- persistent activations ----
            xq_t = persist.tile([P, KC, T], bf16)  # xq^T  [c, t]
            xkv_t = persist.tile([P, KC, T], bf16)  # xkv^T [c, t]
            qt_sb = persist.tile([P, DC, T], bf16)  # Q^T  [d, t]
            kt_sb = persist.tile([P, DC, T], bf16)  # K^T  [d, t]
            v_sb = persist.tile([P, NT, HL, 66], bf16)  # V|1 [t, h, d+1]
            yt_sb = persist.tile([P, DC, T], bf16)  # y^T  [d, t] (normalized)

            # ---- input transposes (XBAR DMA): ONE [512,1024] DMA per
            # granule covers all 8 c-chunks -> out[p, c, t] = x^T[c*128+p, t]
            def emit_xT(dst, src_d, g):
                t0 = g * QW
                nc.sync.dma_start(
                    dst[:, :, t0 : t0 + QW],
                    src_d[t0 : t0 + QW, :],
                    transpose=True,
                )

            emit_xT(xq_t, xq_d, 0)
            nc.gpsimd.dma_start(wkv_sb, wkv_d[:, :])
            emit_xT(xkv_t, xkv_d, 0)
            emit_xT(xkv_t, xkv_d, 1)
            nc.gpsimd.dma_start(wo_sb, wo_d[:, :])
            for g in range(2, NQ):
                emit_xT(xkv_t, xkv_d, g)
            for g in range(1, NQ):
                emit_xT(xq_t, xq_d, g)

            # ---- kernel-wide PSUM: 2 x [128,1024] (s) + 4 x [128,512] (y)
            ps_s = tc.alloc_tile_pool(name="ps_s", bufs=2, space="PSUM")
            ps_y = tc.alloc_tile_pool(name="ps_y", bufs=4, space="PSUM")

            # ---------- emission helpers ----------
            def vproj_unit(tch):
                def u():
                    pv = ps_y.tile([P, QW], f32, tag="y", name="pv")
                    for c in range(KC):
                        nc.tensor.matmul(
                            pv[:, :CL],
                            xkv_t[:, c, tch * P : (tch + 1) * P],
                            wv_ap(c),
                            start=(c == 0),
                            stop=(c == KC - 1),
                        )
                    nc.vector.tensor_copy(
                        v_sb[:, tch, :, 0:64],
                        pv[:, :CL].rearrange("p (h d) -> p h d", h=HL),
                    )
                    nc.vector.tensor_copy(v_sb[:, tch, :, 64:65], ones4)

                return u

            def kproj_unit(g, dc):
                def u():
                    pp = ps_y.tile([P, QW], f32, tag="y", name="ppk")
                    for c in range(KC):
                        nc.tensor.matmul(
                            pp,
                            wk_ap(c, slice(dc * P, (dc + 1) * P)),
                            xkv_t[:, c, g * QW : (g + 1) * QW],
                            start=(c == 0),
                            stop=(c == KC - 1),
                        )
                    nc.vector.tensor_scalar_add(
                        kt_sb[:, dc, g * QW : (g + 1) * QW],
                        pp,
                        bk_sb[:, dc : dc + 1],
                    )

                return u

            def q_prep_units(tq):
                units = []
                for dc in range(DC):

                    def proj_u(dc=dc):
                        pp = ps_y.tile([P, QW], f32, tag="y", name="ppq")
                        for c in range(KC):
                            nc.tensor.matmul(
                                pp,
                                wq_ap(c, slice(dc * P, (dc + 1) * P)),
                                xq_t[:, c, tq * QW : (tq + 1) * QW],
                                start=(c == 0),
                                stop=(c == KC - 1),
                            )
                        nc.vector.tensor_scalar_add(
                            qt_sb[:, dc, tq * QW : (tq + 1) * QW],
                            pp,
                            bq_sb[:, dc : dc + 1],
                        )

                    units.append(proj_u)
                return units

            out_po = out_d.rearrange("(k f p) c -> k p f c", p=P, f=4)
            out_pq = out_d.rearrange("(t p) c -> t p c", p=P)

            def po_units(tq):
                units = []
                state = {}
                for ts_ in range(4):
                    tch = tq * 4 + ts_
                    for co in range(2):

                        def u(tch=tch, ts_=ts_, co=co):
                            if "o" not in state:
                                state["o"] = outst.tile([P, 4, C], bf16, tag="o", name="o_st")
                            po = ps_y.tile([P, QW], f32, tag="y", name="po")
                            for dc in range(DC):
                                nc.tensor.matmul(
                                    po,
                                    yt_sb[:, dc, tch * P : (tch + 1) * P],
                                    wo_ap(dc, slice(co * QW, (co + 1) * QW)),
                                    start=(dc == 0),
                                    stop=(dc == DC - 1),
                                )
                            nc.vector.tensor_copy(
                                state["o"][:, ts_, co * QW : (co + 1) * QW], po
                            )
                            if tq == NQ - 1:
                                if co == 1:
                                    nc.sync.dma_start(
                                        out_pq[tch], state["o"][:, ts_, :]
                                    )
                            elif ts_ == 3 and co == 1:
                                nc.sync.dma_start(out_po[tq], state["o"])

                        units.append(u)
                return units

            # phase-A prep as a streamable queue: per granule g (512 tok):
            # 4 V-proj chunks + 2 K-proj halves
            prep_q = []
            for g in range(NQ):
                for ts_ in range(4):
                    prep_q.append(vproj_unit(g * 4 + ts_))
                for dc in range(DC):
                    prep_q.append(kproj_unit(g, dc))

            # ---- phase B: attention passes per (tq, head-pair) ----
            y_tiles = {}
            e_tiles = {}

            def emit_sexp(k, hc, tk):
                sp = ps_s.tile([P, 2 * QW], f32, tag="s", name="sp")
                for hh in range(2):
                    nc.tensor.matmul(
                        sp[:, hh * QW : (hh + 1) * QW],
                        kt_sb[hh * 64 : (hh + 1) * 64, hc, tk * P : (tk + 1) * P],
                        qt_sb[hh * 64 : (hh + 1) * 64, hc, k * QW : (k + 1) * QW],
                        start=True,
                        stop=True,
                        tile_position=(hh * 64, 0),
                    )
                e2 = ework.tile([P, 2 * QW], bf16, tag="e", name="e2")
                nc.scalar.activation(e2, sp, Exp, scale=0.125)
                e_tiles[(k, hc, tk)] = e2

            def emit_y(k, hc, tk):
                if (k, hc) not in y_tiles:
                    y_tiles[(k, hc)] = [
                        ps_y.tile([65, QW], f32, tag="y", name=f"y_ps{i}")
                        for i in range(2)
                    ]
                y_pair = y_tiles[(k, hc)]
                e2 = e_tiles.pop((k, hc, tk))
                for hh in range(2):
                    h = 2 * hc + hh
                    nc.tensor.matmul(
                        y_pair[hh],
                        v_sb[:, tk, h, :65],
                        e2[:, hh * QW : (hh + 1) * QW],
                        start=(tk == 0),
                        stop=(tk == NT - 1),
                    )

            def emit_norm(k, hc):
                y_pair = y_tiles.pop((k, hc))
                recr = norm2.tile([P, 2, QW], bf16, tag="recr")
                with nc.allow_low_precision(reason="softmax denom reciprocal"):
                    for hh in range(2):
                        nc.vector.reciprocal(
                            recr[64:65, hh, :], y_pair[hh][64:65, :]
                        )
                rbp = ps_s.tile([P, 2 * QW], f32, tag="s", name="rbp")
                for hh in range(2):
                    nc.tensor.matmul(
                        rbp[0:64, hh * QW : (hh + 1) * QW],
                        onesb[64:65, :],
                        recr[64:65, hh, :],
                        start=True,
                        stop=True,
                        tile_position=(64, 0),
                        skip_group_check=True,
                    )
                rbs = norm2.tile([P, 2 * QW], f32, tag="rbs")
                nc.vector.tensor_copy(rbs[0:64, :], rbp[0:64, :])
                for hh in range(2):
                    rb_h = rbs[0:64, hh * QW : (hh + 1) * QW]
                    if hh == 0:
                        nc.vector.tensor_mul(
                            out=yt_sb[0:64, hc, k * QW : (k + 1) * QW],
                            in0=y_pair[hh][0:64, :],
                            in1=rb_h,
                        )
                    else:
                        yst = norm2.tile([64, QW], bf16, tag="yst")
                        nc.vector.tensor_mul(
                            out=yst, in0=y_pair[hh][0:64, :], in1=rb_h
                        )
                        nc.sync.dma_start(
                            yt_sb[64:128, hc, k * QW : (k + 1) * QW], yst
                        )

            passes = [(k, hc) for k in range(NQ) for hc in range(DC)]
            unit_q = []
            yq = []
            # Q-prep for tq0 first (xq g0 is the first DMA), then granule-0
            # prep so S(0,0,0) has kt/v chunk 0
            for u in q_prep_units(0):
                u()
            for _ in range(6):
                prep_q.pop(0)()

            for pi, (k, hc) in enumerate(passes):
                if hc == 0 and k + 1 < NQ:
                    unit_q.extend(q_prep_units(k + 1))
                for tk in range(NT):
                    emit_sexp(k, hc, tk)
                    yq.append((k, hc, tk))
                    if len(yq) > LAG:
                        emit_y(*yq.pop(0))
                    if tk == 1 and pi >= 1:
                        pk, phc = passes[pi - 1]
                        while yq and yq[0][:2] == (pk, phc):
                            emit_y(*yq.pop(0))
                        emit_norm(pk, phc)
                        if hc == 0 and k >= 1:
                            unit_q.extend(po_units(k - 1))
                    # stream phase-A prep ahead of need during the first pass
                    if prep_q:
                        prep_q.pop(0)()
                        if tk % 2 == 0 and prep_q and pi == 0:
                            prep_q.pop(0)()
                    elif unit_q:
                        unit_q.pop(0)()
            while unit_q:
                unit_q.pop(0)()
            while yq:
                emit_y(*yq.pop(0))
            emit_norm(NQ - 1, DC - 1)
            for u in po_units(NQ - 1):
                u()

            ps_y.release()
            ps_s.release()

    nc.compile()
    return nc


def _get_nc():
    if "nc" not in _CACHE:
        _CACHE["nc"] = _build()
    return _CACHE["nc"]


def _shard_inputs(x_q, x_kv, Wq, bq, Wkv, bkv, Wo):
    import ml_dtypes

    bf16 = ml_dtypes.bfloat16

    def pack_proj(W):  # [C, CL] -> [128, 8*256] in (kc, d) order
        return W.reshape(8, P, CL).transpose(1, 0, 2).reshape(P, 8 * CL)

    in_maps = []
    for core in range(NCORES):
        b = core // TPG
        g = core % TPG
        cols = slice(g * CL, (g + 1) * CL)
        wo_loc = Wo[g * CL : (g + 1) * CL, :]  # [256, 1024]
        wblob1 = np.concatenate(
            [
                pack_proj(Wq[:, cols]),
                bq[cols].reshape(2, P).T,
                bkv[:C][cols].reshape(2, P).T,
            ],
            axis=1,
        )
        wkvblob = np.concatenate(
            [
                pack_proj(Wkv[:, :C][:, cols]),
                pack_proj(Wkv[:, C:][:, cols]),
            ],
            axis=1,
        )
        woblob = wo_loc.reshape(2, P, C).transpose(1, 0, 2).reshape(P, 2 * C)
        in_maps.append(
            {
                "xq": np.ascontiguousarray(x_q[b]).astype(bf16),
                "xkv": np.ascontiguousarray(x_kv[b]).astype(bf16),
                "wb1": np.ascontiguousarray(wblob1).astype(bf16),
                "wkv": np.ascontiguousarray(wkvblob).astype(bf16),
                "wo": np.ascontiguousarray(woblob).astype(bf16),
            }
        )
    return in_maps


def kernel(x_q, x_kv, Wq, bq, Wkv, bkv, Wo, bo):
    from concourse.bass_utils import run_bass_kernel_spmd

    x_q = np.asarray(x_q, dtype=np.float32)
    x_kv = np.asarray(x_kv, dtype=np.float32)
    Wq = np.asarray(Wq, dtype=np.float32)
    bq = np.asarray(bq, dtype=np.float32)
    Wkv = np.asarray(Wkv, dtype=np.float32)
    bkv = np.asarray(bkv, dtype=np.float32)
    Wo = np.asarray(Wo, dtype=np.float32)
    bo = np.asarray(bo, dtype=np.float32)

    nc = _get_nc()
    in_maps = _shard_inputs(x_q, x_kv, Wq, bq, Wkv, bkv, Wo)
    res = run_bass_kernel_spmd(nc, in_maps, core_ids=list(range(NCORES)))

    # host-side gather: sum tensor-parallel partials; add exact bias terms
    bias_full = bkv[C:] @ Wo + bo  # v-bias through Wo, plus output bias
    out = np.zeros((B, T, C), dtype=np.float32)
    for core in range(NCORES):
        out[core // TPG] += np.asarray(res.results[core]["out"]).astype(np.float32)
    out += bias_full[None, None, :]
    return out
